# revision 4
# baseline (speedup 1.0000x reference)
"""Trainium2 Bass kernel for HDSLinear (gumbel top-2-of-4 masked linear).

Strategy (column-parallel over out_features, 8 cores x 512 cols):
  - Contraction order d' is gt-major: k = gt*4 + m, row d' = k*128 + p,
    original d = (gt*128 + p)*4 + m. Both x and the weight-side tensors
    use this order, so the mask math lands directly in the transposed
    [d', o] layout the matmul wants and phase-1 produces k-tiles in
    SEQUENTIAL k order (all 4 m-planes of a gt at once).
  - Phase 1 (mask gen) is chunked per (gt, o-half): one 1MB DMA brings
    scores+noise f32, one 0.25MB DMA brings weights (pre-cast bf16 on
    host -- phase 2 is bf16 anyway so precision is identical). Gumbel
    chain on ACT, y-subs + 3 compares + rank/keep on DVE, 3 compares +
    2 adds on Pool. Compare outputs and rank sums are bf16 (values are
    exact small integers) for DVE 2x modes and smaller tiles.
  - Phase 2 consumes wk per-gt tiles in k order, k-MAJOR within each
    pair of s-blocks (8 psum banks = 4 ot x {a,b}); during phase-1 the
    PE gets 8 matmuls per produced k-tile, so mask production is hidden
    under the matmul stream instead of serializing in front of it.
  - Drains: psum + per-partition bias + bf16 cast via ACT (Identity,
    bias AP) for the a-blocks and DVE tensor_scalar_add for b-blocks,
    so drains never stall the PE at pair boundaries.
  - Output written transposed [O_SH, S_TOT] bf16; host reassembles.
"""

import os
import sys
import numpy as np

for _p in ("/opt/trn_rl_repo", "/root/.axon_site/_ro/trn_rl_repo"):
    if os.path.isdir(_p) and _p not in sys.path:
        sys.path.insert(0, _p)

import concourse.bass as bass
import concourse.bacc as bacc
import concourse.mybir as mybir
from concourse import tile
from concourse.bass_utils import run_bass_kernel_spmd

F32 = mybir.dt.float32
BF16 = mybir.dt.bfloat16
AF = mybir.ActivationFunctionType
ALU = mybir.AluOpType

B, S, D_IN, D_OUT = 8, 2048, 4096, 4096
N_CORES = 8
S_TOT = B * S                      # 16384
O_SH = D_OUT // N_CORES            # 512 out-features per core
P = 128
EPS = 1e-10

G_T = 8                            # gt blocks (128 groups each)
K_TILES = 32                       # k = gt*4 + m
S_BLK = 512
N_BLK = S_TOT // S_BLK             # 32 s-blocks
N_PAIR = N_BLK // 2
O_T = O_SH // P                    # 4 o-tiles
HALF = 256                         # phase-1 o-half chunk width

LAST_EXEC_NS = None
_CACHED = {}


def _build_nc():
    nc = bacc.Bacc(None, target_bir_lowering=False)
    xt = nc.declare_dram_parameter("xt", [N_BLK * P, K_TILES * S_BLK], BF16,
                                   isOutput=False)
    # per (gt, half): 8 planes (sc m0..3, nu m0..3) x 256, f32
    snp = nc.declare_dram_parameter("snp", [P, G_T * 2, 8, HALF], F32,
                                    isOutput=False)
    # per (gt, half): 4 m-planes x 256, bf16
    wpk = nc.declare_dram_parameter("wpk", [P, G_T * 2, 4, HALF], BF16,
                                    isOutput=False)
    bsh = nc.declare_dram_parameter("bsh", [P, O_T], F32, isOutput=False)
    out = nc.declare_dram_parameter("out", [O_SH, S_TOT], BF16, isOutput=True)

    with tile.TileContext(nc) as tc:
      with tc.tile_pool(name="const", bufs=1) as const:
        wk = [const.tile([P, O_T, S_BLK], BF16, tag=f"wk{g}", name=f"wk{g}")
              for g in range(G_T)]  # [p, m?, ...] -- free dims [4, 512]: m x o
        bias_col = const.tile([P, O_T], F32, tag="bias_col")
        epsb = const.tile([P, 1], F32, tag="epsb")
        nc.any.memset(epsb[:], EPS)
        nc.sync.dma_start(out=bias_col[:], in_=bsh[:, :])

        with (
            tc.tile_pool(name="xb", bufs=2) as xbp,
            tc.tile_pool(name="p1sn", bufs=3) as p1sn,
            tc.tile_pool(name="p1w", bufs=3) as p1w,
            tc.tile_pool(name="p1c", bufs=2) as p1c,
            tc.tile_pool(name="osb", bufs=3) as osbp,
            tc.tile_pool(name="ps", bufs=1, space="PSUM") as ps,
        ):
            # ---- phase 1: mask production, per (gt, o-half) chunk ----
            # Each chunk is OWNED by one engine (DVE or Pool, ~2:1 split)
            # after the shared ACT gumbel stage, so the 18-op tail is a
            # straight line on one engine with no cross-engine sem latency.
            # The pair-0 x prefetch is emitted AFTER the first two chunks'
            # DMAs so mask production is not starved behind the 8MB of x.
            x_tiles = {}

            def emit_x_prefetch():
                for b0, tg in ((0, "xa"), (1, "xb")):
                    t = xbp.tile([P, K_TILES, S_BLK], BF16, tag=tg,
                                 name=f"x{tg}0")
                    for c in range(4):
                        nc.sync.dma_start(
                            out=t[:, c * 8:(c + 1) * 8, :],
                            in_=xt[b0 * P:(b0 + 1) * P,
                                   c * 8 * S_BLK:(c + 1) * 8 * S_BLK])
                    x_tiles[b0] = t

            chunks = [(gt, hh * HALF, HALF) for gt in range(G_T)
                       for hh in range(2)]
            for idx, (gt, o0h, olen) in enumerate(chunks):
                    ci = gt * 2 + o0h // HALF
                    c0 = o0h % HALF
                    if idx == 2:
                        emit_x_prefetch()
                    sn = p1sn.tile([P, 8, olen], F32, tag="sn",
                                   name=f"sn{idx}")
                    wq = p1w.tile([P, O_T, olen], BF16, tag="wq",
                                  name=f"wq{idx}")
                    nc.sync.dma_start(
                        out=sn[:], in_=snp[:, ci, :, c0:c0 + olen])
                    nc.sync.dma_start(
                        out=wq[:], in_=wpk[:, ci, :, c0:c0 + olen])
                    # gumbel: g2 = ln(-ln(u+eps)+eps) in place (ACT);
                    # y = sc - g2 in place (Pool; is_ge/stt are DVE-only)
                    for m in range(4):
                        nc.scalar.activation(sn[:, 4 + m, :], sn[:, 4 + m, :],
                                             AF.Ln, bias=epsb[:])
                        nc.scalar.activation(sn[:, 4 + m, :], sn[:, 4 + m, :],
                                             AF.Ln, bias=epsb[:], scale=-1.0)
                        nc.gpsimd.tensor_sub(sn[:, m, :], sn[:, m, :],
                                             sn[:, 4 + m, :])

                    def cmp(a, b):
                        t = p1c.tile([P, olen], BF16, tag=f"ge{a}{b}",
                                     name=f"ge{a}{b}_{idx}")
                        nc.vector.tensor_tensor(t[:], sn[:, a, :], sn[:, b, :],
                                                ALU.is_ge)
                        return t

                    ge01, ge02, ge03 = cmp(0, 1), cmp(0, 2), cmp(0, 3)
                    ge12, ge13, ge23 = cmp(1, 2), cmp(1, 3), cmp(2, 3)

                    def keep(m, t1, t2, op2, t3, op3, thr, cmp_op, eng_a):
                        a = p1c.tile([P, olen], BF16, tag=f"acc{m}",
                                     name=f"acc{m}_{idx}")
                        eng_a.tensor_tensor(a[:], t1[:], t2[:], op2)
                        nc.vector.tensor_tensor(a[:], a[:], t3[:], op3)
                        nc.vector.scalar_tensor_tensor(
                            wk[gt][:, m, o0h:o0h + olen], a[:], float(thr),
                            wq[:, m, :], cmp_op, ALU.mult)

                    # rank-of-m vs threshold; ties break to lower index
                    keep(0, ge01, ge02, ALU.add, ge03, ALU.add,
                         1.5, ALU.is_ge, nc.gpsimd)
                    keep(1, ge12, ge13, ALU.add, ge01, ALU.subtract,
                         0.5, ALU.is_ge, nc.vector)
                    keep(2, ge23, ge02, ALU.subtract, ge12, ALU.subtract,
                         -0.5, ALU.is_ge, nc.vector)
                    keep(3, ge03, ge13, ALU.add, ge23, ALU.add,
                         1.5, ALU.is_le, nc.gpsimd)

            # ---- phase 2: out[o, s] = (masked W^T x)^T + bias ----
            for pair in range(N_PAIR):
                ba, bb = 2 * pair, 2 * pair + 1
                if pair == 0:
                    xa, xb = x_tiles[0], x_tiles[1]
                else:
                    xa = xbp.tile([P, K_TILES, S_BLK], BF16, tag="xa",
                                  name=f"xa{pair}")
                    xb = xbp.tile([P, K_TILES, S_BLK], BF16, tag="xb",
                                  name=f"xb{pair}")
                    nc.sync.dma_start(out=xa[:], in_=xt[ba * P:(ba + 1) * P, :])
                    nc.sync.dma_start(out=xb[:], in_=xt[bb * P:(bb + 1) * P, :])
                psa = [ps.tile([P, S_BLK], F32, tag=f"psa{ot}", name=f"psa{ot}_{pair}")
                       for ot in range(O_T)]
                psb = [ps.tile([P, S_BLK], F32, tag=f"psb{ot}", name=f"psb{ot}_{pair}")
                       for ot in range(O_T)]
                # k-major: 8 MMs per k-tile -> PE paced to phase-1 production
                for k in range(K_TILES):
                    gt, m = divmod(k, 4)
                    st, en = (k == 0), (k == K_TILES - 1)
                    for ot in range(O_T):
                        lw = wk[gt][:, m, ot * P:(ot + 1) * P]
                        nc.tensor.matmul(psa[ot][:], lw, xa[:, k, :],
                                         start=st, stop=en)
                        nc.tensor.matmul(psb[ot][:], lw, xb[:, k, :],
                                         start=st, stop=en)
                # drain a-side on ACT, b-side on DVE into one [128, 1024]
                # staging tile per ot; single batched out-DMA per ot.
                for ot in range(O_T):
                    o_sb = osbp.tile([P, 2 * S_BLK], BF16, tag="osb",
                                     name=f"osb{pair}_{ot}")
                    nc.scalar.activation(
                        o_sb[:, 0:S_BLK], psa[ot][:], AF.Identity,
                        bias=bias_col[:, ot:ot + 1])
                    nc.vector.tensor_scalar_add(
                        o_sb[:, S_BLK:2 * S_BLK], psb[ot][:],
                        bias_col[:, ot:ot + 1])
                    nc.sync.dma_start(
                        out=out[ot * P:(ot + 1) * P,
                                ba * S_BLK:(ba + 2) * S_BLK],
                        in_=o_sb[:])
    nc.compile()
    _elide_redundant_ldweights(nc)
    _coalesce_pe_sem_updates(nc)
    return nc


def _ldw_key(inst):
    try:
        pap = inst.ins[0]
        ap = getattr(pap, "ap", None)
        key = (getattr(pap, "memref", None), getattr(pap, "offset", None),
               str(ap), str(getattr(pap, "dtype", None)))
        if key[0] is None and key[1] is None:
            return None
        return key
    except Exception:
        return None


def _elide_redundant_ldweights(nc):
    """Delete InstLdweights that reload the stationary already resident in
    the PE array (same AP as the previous load, no sync side effects)."""
    n_del = 0
    for fn in nc.m.functions:
        for bb in fn.blocks:
            insts = bb.instructions
            last_key = None
            to_del = []
            for idx in range(len(insts)):
                inst = insts[idx]
                tn = type(inst).__name__
                if tn == "InstLdweights":
                    key = _ldw_key(inst)
                    if (key is not None and key == last_key
                            and inst.sync_info is None):
                        to_del.append(idx)
                    else:
                        last_key = key
            for idx in reversed(to_del):
                del insts[idx]
            n_del += len(to_del)
    return n_del


def _coalesce_pe_sem_updates(nc):
    """Drop the per-matmul semaphore increment on non-group-final MMs and
    remap every waiter threshold to the next surviving increment at or
    after its old target (conservative). Mutates only if every wait
    remaps exactly, else leaves the module untouched."""
    try:
        mybir_mod = mybir
        insts = []
        for fn in nc.m.functions:
            for bb in fn.blocks:
                insts.extend(bb.instructions)

        upd_pos = {}          # sem_id -> list[(global_pos, inst, keep)]
        for pos, inst in enumerate(insts):
            si = inst.sync_info
            if not si or not si.on_update:
                continue
            for up in si.on_update:
                if up.sync_type != "semaphore":
                    continue
                is_mm = type(inst).__name__ == "InstMatmult"
                if up.update_mode != "sem-inc" or up.update_value != 1:
                    upd_pos.setdefault(up.id, []).append((pos, inst, True))
                    continue
                keep = True
                if is_mm and str(up.ant_name).startswith("PE"):
                    stop = getattr(inst, "stop_tensor_calc", None)
                    if stop is False:
                        keep = False
                upd_pos.setdefault(up.id, []).append((pos, inst, keep))

        cand = {sid: ups for sid, ups in upd_pos.items()
                if any(not k for _, _, k in ups)}
        if not cand:
            return 0

        plans = []
        ok = True
        for sid, ups in cand.items():
            kept_prefix = []
            c = 0
            for _, _, k in ups:
                c += 1 if k else 0
                kept_prefix.append(c)
            total_old = len(ups)
            for inst in insts:
                si = inst.sync_info
                if not si or not si.on_wait:
                    continue
                for wi, wt in enumerate(si.on_wait):
                    if wt.sync_type != "semaphore" or wt.id != sid:
                        continue
                    if wt.wait_mode != "sem-ge-imm" or wt.wait_reg is not None:
                        ok = False
                        break
                    v = wt.wait_value
                    if v == 0:
                        continue
                    if v > total_old:
                        ok = False
                        break
                    j = v - 1
                    while j < total_old and not ups[j][2]:
                        j += 1
                    if j >= total_old:
                        ok = False
                        break
                    plans.append((inst, wi, sid, kept_prefix[j]))
                if not ok:
                    break
            if not ok:
                break
        if not ok:
            return 0

        SyncInfo = mybir_mod.SyncInfo
        SyncWait = mybir_mod.SyncWait
        by_inst = {}
        for inst, wi, sid, nv in plans:
            by_inst.setdefault(id(inst), (inst, []))[1].append((wi, nv))
        for _, (inst, mods) in by_inst.items():
            si = inst.sync_info
            new_waits = []
            mod_map = dict(mods)
            for wi, wt in enumerate(si.on_wait):
                if wi in mod_map:
                    new_waits.append(SyncWait(
                        sync_type=wt.sync_type, id=wt.id,
                        ant_name=wt.ant_name, wait_mode=wt.wait_mode,
                        wait_value=mod_map[wi], wait_reg=wt.wait_reg))
                else:
                    new_waits.append(wt)
            inst.sync_info = SyncInfo(on_wait=new_waits,
                                      on_update=list(si.on_update))
        n_strip = 0
        for sid, ups in cand.items():
            for _, inst, k in ups:
                if k:
                    continue
                si = inst.sync_info
                new_ups = [u for u in si.on_update
                           if not (u.sync_type == "semaphore" and u.id == sid)]
                inst.sync_info = (
                    SyncInfo(on_wait=list(si.on_wait), on_update=new_ups)
                    if (si.on_wait or new_ups) else None)
                n_strip += 1
        return n_strip
    except Exception:
        return 0


def _get_nc():
    if "nc" not in _CACHED:
        _CACHED["nc"] = _build_nc()
    return _CACHED["nc"]


def _prep_inputs(x, weight, bias, scores, noise_u):
    import ml_dtypes
    BF = ml_dtypes.bfloat16
    x = np.asarray(x, dtype=np.float32).reshape(S_TOT, D_IN)
    weight = np.asarray(weight, dtype=np.float32)
    bias = np.asarray(bias, dtype=np.float32)
    scores = np.asarray(scores, dtype=np.float32).reshape(D_OUT, D_IN)
    noise_u = np.asarray(noise_u, dtype=np.float32).reshape(D_OUT, D_IN)

    # x -> xt[blk*128+p, (gt*4+m)*512+s], gt-major contraction order
    xt = (x.reshape(N_BLK, S_BLK, G_T, P, 4)
            .transpose(0, 3, 2, 4, 1)
            .reshape(N_BLK * P, K_TILES * S_BLK)
            .astype(BF))

    def to_pgmo(t):
        # [o_shard, d] -> [p, gt, m, o]  (d = (gt*128+p)*4 + m)
        return (t.reshape(-1, G_T, P, 4).transpose(2, 1, 3, 0))

    in_maps = []
    for j in range(N_CORES):
        o0 = j * O_SH
        sc = to_pgmo(scores[o0:o0 + O_SH])     # [128, 8, 4, 512]
        nu = to_pgmo(noise_u[o0:o0 + O_SH])
        wj = to_pgmo(weight[o0:o0 + O_SH])
        # [p, gt, m, (h, c)] -> [p, gt, h, m, c]
        sc5 = sc.reshape(P, G_T, 4, 2, HALF).transpose(0, 1, 3, 2, 4)
        nu5 = nu.reshape(P, G_T, 4, 2, HALF).transpose(0, 1, 3, 2, 4)
        snp = np.concatenate([sc5, nu5], axis=3)          # [p, gt, h, 8, c]
        wp5 = (wj.reshape(P, G_T, 4, 2, HALF)
                 .transpose(0, 1, 3, 2, 4).astype(BF))    # [p, gt, h, 4, c]
        bcol = np.ascontiguousarray(
            bias[o0:o0 + O_SH].reshape(O_T, P).T)         # [128, 4]
        in_maps.append({
            "xt": xt,
            "snp": np.ascontiguousarray(snp.reshape(P, G_T * 2, 8, HALF)),
            "wpk": np.ascontiguousarray(wp5.reshape(P, G_T * 2, 4, HALF)),
            "bsh": bcol,
        })
    return in_maps


def kernel(x, weight, bias, scores, noise_u):
    global LAST_EXEC_NS
    in_maps = _prep_inputs(x, weight, bias, scores, noise_u)
    nc = _get_nc()
    if os.environ.get("BASS_KERNEL_TIMED", "0") == "1":
        results, exec_ns = _run_timed(nc, in_maps)
        LAST_EXEC_NS = exec_ns
    else:
        res = run_bass_kernel_spmd(nc, in_maps, list(range(N_CORES)),
                                   trace=False)
        LAST_EXEC_NS = res.exec_time_ns
        results = res.results
    outT = np.concatenate(
        [np.asarray(results[j]["out"]) for j in range(N_CORES)], axis=0)
    return np.ascontiguousarray(outT.T).reshape(B, S, D_OUT).astype(np.float32)


def _run_timed(nc, in_maps, n_iters=512):
    """Pipelined repeat timing with device-resident inputs (amortizes the
    per-call dispatch overhead of the axon tunnel)."""
    import time
    import jax
    from jax.sharding import Mesh, PartitionSpec, NamedSharding
    from jax.experimental.shard_map import shard_map
    from concourse import bass2jax, mybir as _mb

    bass2jax.install_neuronx_cc_hook()
    n_cores = len(in_maps)
    partition_name = (nc.partition_id_tensor.name
                      if nc.partition_id_tensor else None)
    in_names, out_names, out_avals = [], [], []
    for alloc in nc.m.functions[0].allocations:
        if not isinstance(alloc, _mb.MemoryLocationSet):
            continue
        name = alloc.memorylocations[0].name
        if alloc.kind == "ExternalInput":
            if name != partition_name:
                in_names.append(name)
        elif alloc.kind == "ExternalOutput":
            out_names.append(name)
            out_avals.append(jax.core.ShapedArray(
                tuple(alloc.tensor_shape), _mb.dt.np(alloc.dtype)))
    n_params = len(in_names)
    all_names = in_names + out_names + ([partition_name] if partition_name else [])

    def _body(*args):
        operands = list(args)
        if partition_name is not None:
            operands.append(bass2jax.partition_id_tensor())
        return tuple(bass2jax._bass_exec_p.bind(
            *operands, out_avals=tuple(out_avals), in_names=tuple(all_names),
            out_names=tuple(out_names), lowering_input_output_aliases=(),
            sim_require_finite=True, sim_require_nnan=True, nc=nc))

    devices = jax.devices()[:n_cores]
    mesh = Mesh(np.array(devices), ("core",))
    spec = PartitionSpec("core")
    n_outs = len(out_names)
    fn = jax.jit(shard_map(_body, mesh=mesh,
                           in_specs=(spec,) * (n_params + n_outs),
                           out_specs=(spec,) * n_outs, check_rep=False),
                 keep_unused=True)
    sh = NamedSharding(mesh, spec)
    ins_dev = [jax.device_put(
        np.concatenate([np.asarray(m[nm]) for m in in_maps], axis=0), sh)
        for nm in in_names]
    zeros_dev = [jax.device_put(
        np.zeros((n_cores * a.shape[0], *a.shape[1:]), a.dtype), sh)
        for a in out_avals]
    outs = fn(*ins_dev, *zeros_dev)     # compile + warm
    jax.block_until_ready(outs)

    def timed_batch(depth):
        t0 = time.perf_counter()
        for _ in range(depth):
            r = fn(*ins_dev, *zeros_dev)
        jax.block_until_ready(r)
        return (time.perf_counter() - t0) / depth, r

    depth = max(64, n_iters // 2)
    times = []
    last = outs
    for i in range(6):
        if i:
            time.sleep(1.5)
        t_b, last = timed_batch(depth)
        times.append(t_b)
    print(f"[kernel] pipelined per-call (depth {depth} x{len(times)}): "
          + ", ".join(f"{t*1e3:.3f}" for t in times)
          + f" ms -> min {min(times)*1e3:.3f} ms", flush=True)
    dt_ns = min(times) * 1e9
    results = [
        {nm: np.asarray(last[i]).reshape(n_cores, *out_avals[i].shape)[c]
         for i, nm in enumerate(out_names)}
        for c in range(n_cores)]
    return results, int(dt_ns)
